# revision 27
# baseline (speedup 1.0000x reference)
"""AttnBlock (GroupNorm -> single-head self-attention -> residual) on 8 TRN2 cores.

Sharding: B=4 batch elements x 2 query-token halves = 8 cores (SPMD, no
collectives).  Each core receives the full (rolled) channel-major batch
element x^T [C=256, HW=4096] in bf16, computes GroupNorm + k/v for all
4096 tokens, and q/scores/attention/out-proj for its 2048-token half.
Odd cores get x rolled by -2048 tokens; attention is permutation-
invariant over keys, so their first 2048 tokens are tokens 2048:4096.

The two big attention matmuls (scores and attn@v) and the softmax-
denominator chain run in fp8-e4m3 with MatmulPerfMode.DoubleRow
(K=256 packed 2-rows-per-PE-cell, 0.5 cycles/row).  Softmax numerators
use exp(s/16 - 2) so es <= ~57 < 240 (TRN fp8e4 max); the constant
offset cancels in the softmax ratio.  Projections are bf16.  PSUM
accumulation chains never interleave (the PE has one open accumulation
context; interleaving corrupts sums).  GroupNorm rstd uses a Quake-
style rsqrt on DVE (bit-trick + 2 Newton steps) so the ACT engine only
ever loads one table set (Exp/Identity).  Layout is channel-major
(tokens on the free axis), all matmuls transpose-free:

  hs^T = GN(x^T)  bf16                    [C, N]
  q^T = Wq^T.T @ hs^T -> fp8              [C, NQ]   (dim1 = ko ktile)
  k^T likewise -> fp8                     [C, N]
  v   = hs^T.T @ Wv^T + bv -> fp8         [N, C]    (row-major)
  S^T = DR(k^T, q^T)                      [N, NQ]   one matmul per m-tile
  es  = exp(S^T/16 - 2) -> fp8 (ACT)
  o^T = DR(v, es) chain                   [C, NQ]
  Z   = DR(ones, es) chain                [16, NQ]  (row 0 used)
  out^T = (Wo^T*2^-0.5).T @ bf16(o^T)     [C, NQ]
  final = x_bf16 * 2^-0.5 + (out^T + bo*2^-0.5 x Z) * (1/Z)
"""

import numpy as np
import ml_dtypes

import concourse.bass as bass
import concourse.tile as tile
from concourse import bacc, mybir
from concourse.bass_utils import run_bass_kernel_spmd

dt = mybir.dt
F32, F32R, BF16, FP8 = dt.float32, dt.float32r, dt.bfloat16, dt.float8e4
U32 = dt.uint32
AF = mybir.ActivationFunctionType
ALU = mybir.AluOpType
DR = mybir.MatmulPerfMode.DoubleRow

P = 128          # partitions
C = 256          # channels
N = 4096         # tokens per batch element (64*64)
NQ = 2048        # query tokens per core
NSTRIP = 512     # query-token strip width
NS = NQ // NSTRIP  # 4 strips
MT = N // P      # 32 key m-tiles
GS = 8           # channels per group (256 / 32 groups)
EPS = 1e-6
ISCALE = 1.0 / 16.0       # attention scale c**-0.5
EOFF = 2.0                # exp offset: es = exp(s/16 - EOFF), cancels in softmax
RS2 = float(2.0 ** -0.5)  # output residual scale
RSQRT_MAGIC = float(np.frombuffer(np.uint32(0x5F3759DF).tobytes(),
                                  dtype=np.float32)[0])

_prog_cache = {}


def _build_nc(with_bias_o=True):
    nc = bacc.Bacc("TRN2", target_bir_lowering=False, debug=False, num_devices=8)

    def inp(name, shape, d=F32):
        return nc.dram_tensor(name, shape, d, kind="ExternalInput").ap()

    xtb_d = inp("xtb", [2, P, N], BF16)    # [c_half, c_in, n] bf16
    bos_d = inp("bosr", [1, C])            # bo * 2^-0.5 (rank-1 via Z)
    w8_d = inp("w8all", [P, 3, 2, C], FP8)  # [ci_in][wq|wk|wv][ci_half][c_out]
    wo_d = inp("woT", [P, 2, C], BF16)      # Wo.T * 2^-0.5, partition-major
    bq_d = inp("bqp", [P, 2])              # [c_out_in, c_out_half]
    bk_d = inp("bkp", [P, 2])
    bv_d = inp("bv2", [P, 2, C])           # bv pre-broadcast, partition-major
    gnw_d = inp("gnw", [P, 2])
    gnb_d = inp("gnb", [P, 2])
    amat_d = inp("amat", [P, P])           # block-diag 8x8 of 1/8
    ones1_d = inp("ones1", [1, P])
    out_d = nc.dram_tensor("out", [2, P, NQ], F32, kind="ExternalOutput").ap()

    with tile.TileContext(nc) as tc:
        with (
            tc.tile_pool(name="singles", bufs=1) as singles,
            tc.tile_pool(name="xpool", bufs=1) as xpool,
            tc.tile_pool(name="hsp", bufs=1) as hsp,
            tc.tile_pool(name="qk", bufs=1) as qk,
            tc.tile_pool(name="vpool", bufs=1) as vpool,
            tc.tile_pool(name="espool", bufs=2) as espool,
            tc.tile_pool(name="small", bufs=2) as small,
            tc.tile_pool(name="zf", bufs=2) as zf,
            tc.tile_pool(name="ps", bufs=2, space="PSUM") as ps,    # 2x2 banks
            tc.tile_pool(name="po", bufs=2, space="PSUM") as po,    # opa/opb
            tc.tile_pool(name="pz", bufs=1, space="PSUM") as pz,    # zp+op2a
            tc.tile_pool(name="pr", bufs=1, space="PSUM") as pr,    # rp+op2b
        ):
            _dmae = [nc.sync, nc.scalar]
            # ---- tiny consts on the scalar queue; x leads the sync queue ----
            gnw = singles.tile([P, 2], F32)
            nc.scalar.dma_start(gnw[:], gnw_d)
            gnb = singles.tile([P, 2], F32)
            nc.scalar.dma_start(gnb[:], gnb_d)
            bq = singles.tile([P, 2], F32)
            nc.scalar.dma_start(bq[:], bq_d)
            bk = singles.tile([P, 2], F32)
            nc.scalar.dma_start(bk[:], bk_d)
            ones1 = singles.tile([1, P], F32R)
            nc.scalar.dma_start(ones1[:], ones1_d.bitcast(F32R))

            # ---- x load (geometric chunks over both hwdge queues) ----
            xtb = xpool.tile([P, 2, N], BF16, tag="xtb")
            for lo, hi in ((0, 512), (512, 1536), (1536, 4096)):
                for t in range(2):
                    _dmae[t].dma_start(xtb[:, t, lo:hi], xtb_d[t, :, lo:hi])
            # amat via gpsimd software DGE (needed ~25us in)
            amat = singles.tile([P, P], F32R)
            nc.gpsimd.dma_start(amat[:], amat_d.bitcast(F32R))
            bosr = singles.tile([1, C], F32R)
            nc.scalar.dma_start(bosr[:], bos_d.bitcast(F32R))

            # ---- weights: packed partition-major, few large descriptors ----
            w8all = singles.tile([P, 3, 2, C], FP8)
            nc.scalar.dma_start(w8all[:], w8_d)
            wo = singles.tile([P, 2, C], BF16)
            nc.scalar.dma_start(wo[:], wo_d)
            bvrep = singles.tile([P, 2, C], F32)
            nc.sync.dma_start(bvrep[:], bv_d)
            wq, wk, wv = w8all[:, 0], w8all[:, 1], w8all[:, 2]
            ones8z = singles.tile([P, 2, 16], FP8)
            nc.vector.memset(ones8z[:], 1.0)
            noff = singles.tile([P, 1], F32)
            nc.vector.memset(noff[:], -EOFF)
            epsap = singles.tile([P, 1], F32)
            nc.vector.memset(epsap[:], EPS)
            magic = singles.tile([P, 2], F32)
            nc.vector.memset(magic[:], RSQRT_MAGIC)

            # ---- GroupNorm stats (per channel, then 8-chan group aggregate) ----
            mv2 = small.tile([P, 4], F32, tag="gnmv")  # [mu_t0 mu_t1 ex2_t0 ex2_t1]
            sts = [small.tile([P, 8, 6], F32, tag="gnst", name=f"gnst{t}")
                   for t in range(2)]
            for sgs in ((0,), (1, 2), (3, 4, 5, 6, 7)):
                for t in range(2):
                    xre = xtb[:, t, :].rearrange("p (s f) -> p s f", f=512)
                    for sg in sgs:
                        nc.vector.bn_stats(sts[t][:, sg, :], xre[:, sg, :])
            for t in range(2):
                mvt = small.tile([P, 2], F32, tag="gnmvt", name=f"gnmvt{t}")
                nc.vector.bn_aggr(mvt[:], sts[t][:])  # [mean, var]
                musq = small.tile([P, 1], F32, tag="gnmusq", name=f"gnmusq{t}")
                nc.vector.tensor_mul(musq[:], mvt[:, 0:1], mvt[:, 0:1])
                nc.vector.tensor_copy(mv2[:, t:t + 1], mvt[:, 0:1])
                nc.vector.tensor_add(mv2[:, 2 + t:3 + t], mvt[:, 1:2], musq[:])
            stats2 = small.tile([P, 4], F32R, tag="gnst2")
            nc.vector.tensor_copy(stats2[:], mv2[:])
            gp = pz.tile([P, 512], F32, tag="pz", name="gnagg")
            nc.tensor.matmul(gp[:, 0:4], amat[:], stats2[:], start=True, stop=True)
            gs = small.tile([P, 4], F32, tag="gnagg2")
            nc.vector.tensor_copy(gs[:], gp[:, 0:4])
            gmusq = small.tile([P, 2], F32, tag="gnmusq2")
            nc.vector.tensor_mul(gmusq[:], gs[:, 0:2], gs[:, 0:2])
            gvar = small.tile([P, 2], F32, tag="gnvar")
            nc.vector.tensor_tensor(gvar[:], gs[:, 2:4], gmusq[:], ALU.subtract)
            # rstd = rsqrt(var + eps): Quake bit-trick + 2 Newton steps (DVE
            # only — keeps the ACT table on the Exp/Identity set throughout)
            vpe = small.tile([P, 2], F32, tag="gnvpe")
            nc.vector.tensor_scalar(vpe[:], gvar[:], epsap[:], None, ALU.add)
            y0 = small.tile([P, 2], F32, tag="gny0")
            nc.vector.tensor_scalar(y0[:].bitcast(U32), vpe[:].bitcast(U32),
                                    1, None, ALU.logical_shift_right)
            nc.vector.tensor_tensor(y0[:].bitcast(U32), magic[:].bitcast(U32),
                                    y0[:].bitcast(U32), ALU.subtract)
            rstd = small.tile([P, 2], F32, tag="gnrstd")
            tnw = small.tile([P, 2], F32, tag="gnnewt")
            for it in range(2):
                sr = y0 if it == 0 else rstd
                nc.vector.tensor_mul(tnw[:], sr[:], sr[:])
                nc.vector.tensor_mul(tnw[:], tnw[:], vpe[:])
                with nc.allow_low_precision(reason="rsqrt newton step"):
                    nc.vector.tensor_scalar(tnw[:], tnw[:], -0.5, 1.5,
                                            ALU.mult, ALU.add)
                nc.vector.tensor_mul(rstd[:], sr[:], tnw[:])
            alpha = small.tile([P, 2], F32, tag="gnalpha")
            nc.vector.tensor_mul(alpha[:], rstd[:], gnw[:])
            atmp = small.tile([P, 2], F32, tag="gnatmp")
            nc.vector.tensor_mul(atmp[:], gs[:, 0:2], alpha[:])
            beta = small.tile([P, 2], F32, tag="gnbeta")
            nc.vector.tensor_tensor(beta[:], gnb[:], atmp[:], ALU.subtract)
            hs = hsp.tile([P, 2, N], FP8, tag="hs")

            def emit_hs_span(lo, hi, on_act):
                for t in range(2):
                    if on_act == 1 or (on_act == 2 and t == 0):
                        nc.scalar.activation(
                            hs[:, t, lo:hi], xtb[:, t, lo:hi],
                            AF.Identity, bias=beta[:, t:t + 1],
                            scale=alpha[:, t:t + 1])
                    else:
                        nc.vector.tensor_scalar(
                            hs[:, t, lo:hi], xtb[:, t, lo:hi],
                            alpha[:, t:t + 1], beta[:, t:t + 1],
                            ALU.mult, ALU.add)

            kT = qk.tile([P, 2, N], FP8, tag="kT")
            qT = qk.tile([P, 2, NQ], FP8, tag="qT")

            def emit_proj(wt, bt, dst, blk, on_act, nm):
                # one 512-token block of a q/k projection for both ch halves
                for ch in range(2):
                    kp = po.tile([P, 512], F32, tag="po",
                                 name=f"pj{nm}_{ch}_{blk}")
                    nc.tensor.matmul(
                        kp[:], wt[:, :, ch * P:(ch + 1) * P],
                        hs[:, :, blk * 512:(blk + 1) * 512],
                        start=True, stop=True, perf_mode=DR)
                    sl = dst[:, ch, blk * 512:(blk + 1) * 512]
                    if on_act:
                        nc.scalar.activation(sl, kp[:], AF.Identity,
                                             bias=bt[:, ch:ch + 1], scale=1.0)
                    else:
                        nc.vector.tensor_scalar(sl, kp[:], bt[:, ch:ch + 1],
                                                None, ALU.add)

            v = vpool.tile([P, MT, C], FP8)

            def emit_vproj():
                for g in range(MT // 2):
                    vp = po.tile([P, 2, NSTRIP // 2], F32, tag="po",
                                 name=f"vp{g}")
                    for i in range(2):
                        m = 2 * g + i
                        nc.tensor.matmul(vp[:, i, :],
                                         hs[:, :, m * P:(m + 1) * P],
                                         wv[:, :, :],
                                         start=True, stop=True, perf_mode=DR)
                    nc.vector.tensor_tensor(v[:, 2 * g:2 * g + 2, :], vp[:],
                                            bvrep[:], ALU.add)

            # ---- attention strips (software-pipelined emission) ----
            es_t = [None] * NS
            opa_t = [None] * NS
            opb_t = [None] * NS
            zp_t = [None] * NS
            zsb_t = [None] * NS
            rz_t = [None] * NS
            osb_t = [None] * NS

            def emit_scores_exp(s):
                ns = slice(s * NSTRIP, (s + 1) * NSTRIP)
                es = espool.tile([P, MT, NSTRIP], FP8, tag="es", name=f"es{s}")
                es_t[s] = es
                for j in range(MT // 2):
                    sp = ps.tile([P, 2, NSTRIP], F32, tag="ps", name=f"sp{s}_{j}")
                    for i in range(2):
                        m = 2 * j + i
                        nc.tensor.matmul(sp[:, i, :],
                                         kT[:, :, m * P:(m + 1) * P],
                                         qT[:, :, ns],
                                         start=True, stop=True, perf_mode=DR)
                    nc.scalar.activation(
                        es[:, 2 * j:2 * j + 2, :].rearrange("p a b -> p (a b)"),
                        sp[:].rearrange("p a b -> p (a b)"),
                        AF.Exp, bias=noff[:], scale=ISCALE)

            def emit_zav(s):
                # three accumulation chains, never interleaved (PE constraint)
                es = es_t[s]
                opa = po.tile([P, NSTRIP], F32, tag="po", name=f"opa{s}")
                opb = po.tile([P, NSTRIP], F32, tag="po", name=f"opb{s}")
                zp = pz.tile([P, 512], F32, tag="pz", name=f"zp{s}")
                osb = small.tile([P, 2, NSTRIP], BF16, tag="osb", name=f"osb{s}")
                opa_t[s], opb_t[s], osb_t[s] = opa, opb, osb
                zp_t[s] = zp
                for j2 in range(MT // 2):
                    nc.tensor.matmul(zp[0:16, 0:NSTRIP], ones8z[:],
                                     es[:, 2 * j2:2 * j2 + 2, :],
                                     start=(j2 == 0), stop=(j2 == MT // 2 - 1),
                                     perf_mode=DR)
                rzf = small.tile([1, NSTRIP], F32, tag="rzf", name=f"rzf{s}")
                rz = small.tile([1, NSTRIP], F32R, tag="rz", name=f"rz{s}")
                rz_t[s] = rz
                with nc.allow_low_precision(reason="~18-bit 1/Z is plenty"):
                    nc.vector.reciprocal_approx_fast(rzf[:], zp[0:1, 0:NSTRIP])
                    nc.vector.tensor_copy(rz[:], rzf[:])
                if with_bias_o:
                    zsb = small.tile([1, NSTRIP], F32R, tag="zsb",
                                     name=f"zsb{s}")
                    zsb_t[s] = zsb
                    with nc.allow_low_precision(reason="denominator copy"):
                        nc.vector.tensor_copy(zsb[:], zp[0:1, 0:NSTRIP])
                for j2 in range(MT // 2):
                    nc.tensor.matmul(opa[:],
                                     v[:, 2 * j2:2 * j2 + 2, 0:P],
                                     es[:, 2 * j2:2 * j2 + 2, :],
                                     start=(j2 == 0), stop=(j2 == MT // 2 - 1),
                                     perf_mode=DR)
                nc.vector.tensor_copy(osb[:, 0, :], opa[:])
                for j2 in range(MT // 2):
                    nc.tensor.matmul(opb[:],
                                     v[:, 2 * j2:2 * j2 + 2, P:2 * P],
                                     es[:, 2 * j2:2 * j2 + 2, :],
                                     start=(j2 == 0), stop=(j2 == MT // 2 - 1),
                                     perf_mode=DR)

            def emit_tail_b(s):
                ns = slice(s * NSTRIP, (s + 1) * NSTRIP)
                osb = osb_t[s]
                nc.vector.tensor_copy(osb[:, 1, :], opb_t[s][:])
                rp = pr.tile([P, 512], F32, tag="pr", name=f"rp{s}")
                nc.tensor.matmul(rp[:, 0:NSTRIP], ones1[:], rz_t[s][:],
                                 start=True, stop=True)
                rzs = small.tile([P, NSTRIP], F32, tag="rzs", name=f"rzs{s}")
                nc.vector.tensor_copy(rzs[:], rp[:, 0:NSTRIP])
                op2a = pz.tile([P, 512], F32, tag="pz", name=f"op2a{s}")
                op2b = pr.tile([P, 512], F32, tag="pr", name=f"op2b{s}")
                for ch, op2 in ((0, op2a), (1, op2b)):
                    for ko in range(2):
                        nc.tensor.matmul(op2[:, 0:NSTRIP],
                                         wo[:, ko, ch * P:(ch + 1) * P],
                                         osb[:, ko, :],
                                         start=(ko == 0),
                                         stop=(ko == 1 and not with_bias_o))
                    if with_bias_o:
                        # rank-1 (bo*2^-0.5) x Z: cancels the later *(1/Z)
                        nc.tensor.matmul(op2[:, 0:NSTRIP],
                                         bosr[0:1, ch * P:(ch + 1) * P],
                                         zsb_t[s][:], start=False, stop=True)
                tt = zf.tile([P, 2, NSTRIP], F32, tag="tt", name=f"tt{s}")
                for ch, op2 in ((0, op2a), (1, op2b)):
                    nc.vector.tensor_tensor(tt[:, ch, :], op2[:, 0:NSTRIP],
                                            rzs[:], ALU.mult)
                fin = zf.tile([P, 2, NSTRIP], F32, tag="fin", name=f"fin{s}")
                nc.vector.scalar_tensor_tensor(
                    out=fin[:], in0=xtb[:, :, ns], scalar=RS2, in1=tt[:],
                    op0=ALU.mult, op1=ALU.add)
                for t in range(2):
                    nc.sync.dma_start(out_d[t, :, ns], fin[:, t, :])

            # ACT = q0 cast + pure exp stream; all other casts on DVE.
            # k-cast blocks trail the scores consumption with margin.
            emit_hs_span(0, 512, 2)
            emit_proj(wq, bq, qT, 0, on_act=True, nm="q")
            emit_proj(wk, bk, kT, 0, on_act=False, nm="k")
            emit_hs_span(512, 2048, 0)
            for blk in range(1, 4):
                emit_proj(wk, bk, kT, blk, on_act=False, nm="k")
            emit_hs_span(2048, 4096, 0)
            for blk in range(4, 8):
                emit_proj(wk, bk, kT, blk, on_act=False, nm="k")
            emit_scores_exp(0)
            for blk in range(1, NQ // 512):
                emit_proj(wq, bq, qT, blk, on_act=False, nm="q")
            emit_scores_exp(1)
            emit_vproj()
            for s in range(NS):
                emit_zav(s)
                if s + 2 < NS:
                    emit_scores_exp(s + 2)
                emit_tail_b(s)

    nc.finalize()
    return nc


def _get_nc(with_bias_o=True):
    key = f"nc{int(with_bias_o)}"
    if key not in _prog_cache:
        _prog_cache[key] = _build_nc(with_bias_o)
    return _prog_cache[key]


def _make_in_maps(x, gn_weight, gn_bias, Wq, bq, Wk, bk, Wv, bv, Wo, bo):
    x = np.asarray(x, dtype=np.float32)
    f32 = lambda a: np.ascontiguousarray(np.asarray(a, dtype=np.float32))
    b16 = lambda a: np.ascontiguousarray(
        np.asarray(a, dtype=np.float32).astype(ml_dtypes.bfloat16))

    def packT(b_vec):  # [256] -> [128, 2] (c_out_in, c_out_half)
        return np.ascontiguousarray(f32(b_vec).reshape(2, P).T)

    amat = np.zeros((P, P), np.float32)
    for g in range(P // GS):
        amat[g * GS:(g + 1) * GS, g * GS:(g + 1) * GS] = 1.0 / GS

    fp8c = lambda a: np.asarray(
        np.asarray(a, dtype=np.float32).astype(ml_dtypes.float8_e4m3))
    # [P, 3, 2, C]: partition-major stack of Wq.T, Wk.T, Wv.T (ktile = ci half)
    w8all = np.stack([fp8c(np.asarray(W).T).reshape(2, P, C).transpose(1, 0, 2)
                      for W in (Wq, Wk, Wv)], axis=1)
    common = {
        "w8all": np.ascontiguousarray(w8all),
        "woT": np.ascontiguousarray(
            b16(np.asarray(Wo, dtype=np.float32).T * RS2)
            .reshape(2, P, C).transpose(1, 0, 2)),
        "bqp": packT(bq),
        "bkp": packT(bk),
        "bv2": np.ascontiguousarray(
            np.broadcast_to(f32(bv).reshape(1, 1, C), (P, 2, C))),
        "bosr": np.ascontiguousarray(f32(bo).reshape(1, C) * RS2),
        "gnw": packT(gn_weight),
        "gnb": packT(gn_bias),
        "amat": amat,
        "ones1": np.ones((1, P), np.float32),
    }

    in_maps = []
    for core in range(8):
        b, half = core // 2, core % 2
        xt = x[b].reshape(C, N)
        if half:
            xt = np.roll(xt, -NQ, axis=1)
        in_maps.append({
            "xtb": np.ascontiguousarray(
                xt.astype(ml_dtypes.bfloat16)).reshape(2, P, N),
            **common,
        })
    return in_maps


def _assemble(results, B):
    out = np.empty((B, C, N), np.float32)
    for core in range(2 * B):
        b, half = core // 2, core % 2
        out[b, :, half * NQ:(half + 1) * NQ] = results[core]["out"].reshape(C, NQ)
    return out.reshape(B, C, 64, 64)


def kernel(x, gn_weight, gn_bias, Wq, bq, Wk, bk, Wv, bv, Wo, bo):
    x = np.asarray(x, dtype=np.float32)
    in_maps = _make_in_maps(x, gn_weight, gn_bias, Wq, bq, Wk, bk, Wv, bv, Wo, bo)
    nc = _get_nc(with_bias_o=bool(np.any(np.asarray(bo) != 0)))
    res = run_bass_kernel_spmd(nc, in_maps, list(range(8)))
    return _assemble(res.results, x.shape[0])
